# revision 6
# baseline (speedup 1.0000x reference)
"""Trainium2 Bass kernel for BoundaryLoss (data-parallel over batch).

Math (per batch sample b):
  mask  = boundary mask of target = (maxpool5x5(t) != minpool5x5(t)) with
          cv2-style clipped windows (OOB ignored).  Equals the reference's
          per-class dilate/erode union because a 5x5 window is non-uniform
          iff some class boundary passes through it.
  ce    = logsumexp_c(pred) - pred[t]
  wsum  = sum(mask * ce);  msum = sum(mask)
  per_sample = msum > 0 ? wsum/max(msum,1) : wsum/(H*W);  out = mean_b

Device algorithm (one sample per core):
  - mask pipeline in "layout A" [128, (4, 512)]: partition p = row g*128+p.
    Horizontal 5-max/min via 3 shifted tensor_tensor ops (DVE), PE-transpose
    128x128 blocks, vertical pools the same way in transposed space,
    compare, PE-transpose the mask back.  tt2 = (t+1)*mask.
  - tt2/mask are bounced through DRAM into "layout B" [128, (4 rows, 512)]
    (partition p = rows 4p..4p+3) which gives 8KB-contiguous DMA runs for
    the 21 MB pred stream (DMA is the roofline for this kernel).
  - per class c: e_c = exp(pred_c) on ACT (fp16 out);
      S += e_c             via identity-matmul PSUM accumulation (TensorE)
      G += (tt2==c+1)*e_c  eq on DVE 4x + mult on DVE 2x + identity matmul
    After the loop S = sum_c exp(pred_c), G = mask * exp(pred[t]).
  - sum(mask*lse) = reduce(mask * ln(S)); sum(mask*pred[t]) =
    reduce(ln(G - mask + 1)); partition-reduce via ones-matmul; DMA [1,8].
Host combines the 8 (w1, l2, msum) triples.
"""

import numpy as np

B = 8
C = 21
H = 512
W = 512
N_CORES = 8
CHUNK = 2  # pred planes per DMA
PW = 520  # padded width of pooling buffers; data cols [2, 514)
G4 = 4  # row groups (H = G4 * 128)

_CACHE = {}


def _build_nc():
    from contextlib import ExitStack

    import concourse.bacc as bacc
    import concourse.tile as tile
    from concourse import mybir
    from concourse.masks import make_identity

    dt = mybir.dt
    Alu = mybir.AluOpType
    Act = mybir.ActivationFunctionType

    nc = bacc.Bacc("TRN2", target_bir_lowering=False, debug=False,
                   num_devices=N_CORES)

    pred = nc.dram_tensor("pred", [C, H, W], dt.float32, kind="ExternalInput")
    target = nc.dram_tensor("target", [H, W], dt.int32, kind="ExternalInput")
    out = nc.dram_tensor("out", [1, 8], dt.float32, kind="ExternalOutput")
    # DRAM bounce buffers for layout A -> layout B relayout
    tt2_dram = nc.dram_tensor("tt2_dram", [H, W], dt.float16)
    mask_dram = nc.dram_tensor("mask_dram", [H, W], dt.float16)

    with tile.TileContext(nc) as tc, ExitStack() as ctx:
        consts = ctx.enter_context(tc.tile_pool(name="consts", bufs=1))
        keep = ctx.enter_context(tc.tile_pool(name="keep", bufs=1))
        mp = ctx.enter_context(tc.tile_pool(name="maskpool", bufs=1))
        ms = ctx.enter_context(tc.tile_pool(name="maskscratch", bufs=1))
        ppool = ctx.enter_context(tc.tile_pool(name="pp", bufs=3))
        epool = ctx.enter_context(tc.tile_pool(name="ep", bufs=3))
        qpool = ctx.enter_context(tc.tile_pool(name="qp", bufs=4))
        opool = ctx.enter_context(tc.tile_pool(name="op", bufs=4))
        fin = ctx.enter_context(tc.tile_pool(name="fin", bufs=1))
        jpool = ctx.enter_context(tc.tile_pool(name="jp", bufs=2))

        ident = consts.tile([128, 128], dt.float16)
        make_identity(nc, ident)
        ones = consts.tile([128, 1], dt.float32)
        nc.gpsimd.memset(ones, 1.0)
        warm = consts.tile([128, 512], dt.float16)
        nc.gpsimd.memset(warm, 0.0)
        st_w1 = consts.tile([128, 1], dt.float32)
        st_l2 = consts.tile([128, 1], dt.float32)
        st_m = consts.tile([128, 1], dt.float32)

        # layout-B mask products consumed by the class loop / finals
        tt2b = keep.tile([128, G4, W], dt.float16)
        maskb = keep.tile([128, G4, W], dt.float16)

        # ---------------- target load + PE warmup ----------------
        t32 = mp.tile([128, G4, W], dt.int32)
        nc.sync.dma_start(
            out=t32, in_=target.ap().rearrange("(g p) w -> p g w", p=128))

        with tc.tile_pool(name="warmpsum", bufs=1, space="PSUM") as wps:
            wp = wps.tile([128, 512], dt.float32)
            for _ in range(10):
                nc.tensor.matmul(wp, ident, warm, start=True, stop=True)

        # ---------------- boundary mask (layout A) ----------------
        with tc.tile_pool(name="mpsum", bufs=2, space="PSUM") as mps:
            xmax = mp.tile([128, G4, PW], dt.float16, tag="xmax")
            xmin = mp.tile([128, G4, PW], dt.float16, tag="xmin")
            for t in (xmax, xmin):
                s = -1.0 if t is xmax else 99.0
                nc.gpsimd.memset(t[:, :, 0:2], s)
                nc.gpsimd.memset(t[:, :, 2 + W:PW], s)
            nc.vector.tensor_copy(out=xmax[:, :, 2:2 + W], in_=t32)
            nc.vector.tensor_copy(out=xmin[:, :, 2:2 + W], in_=t32)

            def pool5(src, op, dst, engine):
                # dst[:, :, c] = op over src[:, :, c : c+5] (c = padded-2)
                m2 = ms.tile([128, G4, PW], dt.float16, tag="m2")
                m4 = ms.tile([128, G4, PW], dt.float16, tag="m4")
                engine.tensor_tensor(
                    out=m2[:, :, 0:PW - 1],
                    in0=src[:, :, 0:PW - 1], in1=src[:, :, 1:PW], op=op)
                engine.tensor_tensor(
                    out=m4[:, :, 0:PW - 3],
                    in0=m2[:, :, 0:PW - 3], in1=m2[:, :, 2:PW - 1], op=op)
                engine.tensor_tensor(
                    out=dst,
                    in0=m4[:, :, 0:W], in1=src[:, :, 4:4 + W], op=op)

            hx = mp.tile([128, G4, W], dt.float16, tag="hx")
            hn = mp.tile([128, G4, W], dt.float16, tag="hn")
            pool5(xmax, Alu.max, hx, nc.vector)
            pool5(xmin, Alu.min, hn, nc.vector)

            # transpose hx/hn: xt[p', q, 2+r] = h[r, q*128+p']
            xt = mp.tile([128, G4, PW], dt.float16, tag="xt")
            xnt = mp.tile([128, G4, PW], dt.float16, tag="xnt")
            for t in (xt, xnt):
                s = -1.0 if t is xt else 99.0
                nc.gpsimd.memset(t[:, :, 0:2], s)
                nc.gpsimd.memset(t[:, :, 2 + W:PW], s)
            for src, dst in ((hx, xt), (hn, xnt)):
                for q in range(4):
                    tq = mps.tile([128, 512], dt.float16, tag="tq")
                    for g in range(4):
                        nc.tensor.transpose(
                            tq[:, g * 128:(g + 1) * 128],
                            src[:, g, q * 128:(q + 1) * 128], ident)
                    nc.scalar.copy(out=dst[:, q, 2:2 + W], in_=tq)

            # vertical pools + compare in transposed space
            # reuse the dead H-pool slots (hx/hn fully consumed by the
            # forward transposes, which xt/xnt-readiness already orders)
            vx = mp.tile([128, G4, W], dt.float16, tag="hx")
            vn = mp.tile([128, G4, W], dt.float16, tag="hn")
            pool5(xt, Alu.max, vx, nc.vector)
            pool5(xnt, Alu.min, vn, nc.vector)
            maskt = mp.tile([128, G4, W], dt.float16, tag="maskt")
            nc.vector.tensor_tensor(out=maskt, in0=vx, in1=vn,
                                    op=Alu.not_equal)

            # msum via ACT copy with accumulate
            junk_m = ms.tile([128, G4, W], dt.float16, tag="junkm")
            nc.scalar.activation(out=junk_m, in_=maskt, func=Act.Copy,
                                 accum_out=st_m)

            # transpose mask back to row layout A
            mask_a = mp.tile([128, G4, W], dt.float16, tag="maska")
            for g in range(4):
                tg = mps.tile([128, 512], dt.float16, tag="tq")
                for q in range(4):
                    nc.tensor.transpose(
                        tg[:, q * 128:(q + 1) * 128],
                        maskt[:, q, g * 128:(g + 1) * 128], ident)
                nc.scalar.copy(out=mask_a[:, g, :], in_=tg)

            # tt2 = (t+1) * mask  (layout A)
            tt2a = mp.tile([128, G4, W], dt.float16, tag="tt2a")
            nc.vector.scalar_tensor_tensor(
                out=tt2a, in0=xmax[:, :, 2:2 + W], scalar=1.0, in1=mask_a,
                op0=Alu.add, op1=Alu.mult)

            # relayout A -> B through DRAM
            nc.sync.dma_start(
                out=tt2_dram.ap().rearrange("(g p) w -> p g w", p=128),
                in_=tt2a)
            nc.sync.dma_start(
                out=mask_dram.ap().rearrange("(g p) w -> p g w", p=128),
                in_=mask_a)
            nc.sync.dma_start(
                out=tt2b,
                in_=tt2_dram.ap().rearrange("(p r) w -> p r w", p=128))
            nc.sync.dma_start(
                out=maskb,
                in_=mask_dram.ap().rearrange("(p r) w -> p r w", p=128))

        # ---------------- class loop (layout B) ----------------
        with tc.tile_pool(name="sgpsum", bufs=1, space="PSUM") as sgp:
            s_ps = sgp.tile([128, G4, W], dt.float32, tag="s")
            g_ps = sgp.tile([128, G4, W], dt.float32, tag="g")

            n_chunks = (C + CHUNK - 1) // CHUNK
            for k in range(n_chunks):
                c0 = k * CHUNK
                nct = min(CHUNK, C - c0)
                p_t = ppool.tile([128, nct, G4, W], dt.float32, tag="p")
                nc.sync.dma_start(
                    out=p_t,
                    in_=pred.ap()[c0:c0 + nct].rearrange(
                        "c (p r) w -> p c r w", p=128))
                e_t = epool.tile([128, nct, G4, W], dt.float16, tag="e")
                nc.scalar.activation(out=e_t, in_=p_t, func=Act.Exp)
                for i in range(nct):
                    c = c0 + i
                    eq_t = qpool.tile([128, G4, W], dt.float16, tag="q")
                    nc.vector.tensor_scalar(
                        out=eq_t, in0=tt2b, scalar1=float(c + 1), scalar2=None,
                        op0=Alu.is_equal)
                    o_t = opool.tile([128, G4, W], dt.float16, tag="o")
                    nc.vector.tensor_tensor(
                        out=o_t, in0=eq_t, in1=e_t[:, i], op=Alu.mult)
                    for j in range(4):
                        nc.tensor.matmul(
                            s_ps[:, j, :], ident, e_t[:, i, j, :],
                            start=(c == 0), stop=(c == C - 1))
                        nc.tensor.matmul(
                            g_ps[:, j, :], ident, o_t[:, j, :],
                            start=(c == 0), stop=(c == C - 1))

            # ---------------- finals (two independent chains) ----------------
            l1 = fin.tile([128, G4, W], dt.float32)
            nc.scalar.activation(out=l1, in_=s_ps, func=Act.Ln)
            v = fin.tile([128, G4, W], dt.float32)
            nc.vector.scalar_tensor_tensor(
                out=v, in0=maskb, scalar=-1.0, in1=g_ps,
                op0=Alu.mult, op1=Alu.add)

        j1 = jpool.tile([128, G4, W], dt.float32, tag="junk")
        nc.vector.scalar_tensor_tensor(
            out=j1, in0=l1, scalar=0.0, in1=maskb,
            op0=Alu.add, op1=Alu.mult, accum_out=st_w1)
        j2 = jpool.tile([128, G4, W], dt.float32, tag="junk")
        nc.scalar.activation(out=j2, in_=v, func=Act.Ln, bias=1.0,
                             accum_out=st_l2)

        # partition reductions (independent tiny fp32 matmuls)
        with tc.tile_pool(name="rpsum", bufs=1, space="PSUM") as rp:
            red = rp.tile([1, 8], dt.float32)
            nc.tensor.matmul(red[:, 0:1], ones, st_w1, start=True, stop=True)
            nc.tensor.matmul(red[:, 1:2], ones, st_l2, start=True, stop=True)
            nc.tensor.matmul(red[:, 2:3], ones, st_m, start=True, stop=True)
            outsb = consts.tile([1, 8], dt.float32)
            nc.vector.memset(outsb, 0.0)
            nc.vector.tensor_copy(out=outsb[:, 0:3], in_=red[:, 0:3])
        nc.sync.dma_start(out=out.ap(), in_=outsb)

    nc.compile()
    return nc


def get_nc():
    if "nc" not in _CACHE:
        _CACHE["nc"] = _build_nc()
    return _CACHE["nc"]


def _combine(outs):
    """outs: list of per-core [1,8] float32 -> scalar loss."""
    per_sample = []
    for o in outs:
        w1, l2, msum = float(o[0, 0]), float(o[0, 1]), float(o[0, 2])
        wsum = w1 - l2
        if msum > 0:
            per_sample.append(wsum / max(msum, 1.0))
        else:
            per_sample.append(wsum / float(H * W))
    return np.float32(np.mean(per_sample))


def kernel(pred, target):
    from concourse.bass_utils import run_bass_kernel_spmd

    pred = np.ascontiguousarray(pred, dtype=np.float32)
    target = np.ascontiguousarray(target, dtype=np.int32)
    assert pred.shape == (B, C, H, W) and target.shape == (B, H, W)

    nc = get_nc()
    in_maps = [{"pred": pred[b], "target": target[b]} for b in range(B)]
    res = run_bass_kernel_spmd(nc, in_maps, core_ids=list(range(N_CORES)))
    outs = [res.results[b]["out"] for b in range(B)]
    return np.asarray(_combine(outs), dtype=np.float32)
